# revision 9
# baseline (speedup 1.0000x reference)
"""APPNP propagation (10 iterations of h <- 0.9*A@h + 0.1*x) on 8 TRN2 NeuronCores.

Strategy (row sharding + sectioned ELLPACK via dma_gather):
  - Nodes are permuted so rows are grouped by degree into blocks of 128; each
    core owns NB blocks (snake assignment balances per-core edge counts).
  - The gather table holds h in bf16 padded to 256B row stride ([NPAD, 128]
    bf16, only cols 0..63 used) because dma_gather requires a 256B-multiple
    row stride; gathering only the 128B payload halves DMA time vs f32.
  - dma_gather indices are int16, so the table is split into sections of
    SEC<=32768 rows; every row's neighbor list is bucketed per section and
    padded per (slot-chunk, section) to a uniform degree g so one gather +
    one multiply + one 4D strided reduce handles a whole chunk-section.
  - Per iteration: gathers (Pool/SDMA) -> weight multiply + segmented reduce
    (DVE) -> per-chunk combine of section partials -> residual -> AllGather
    of bf16 shards into the next table. Output written f32 on the last
    iteration and unpermuted on the host.
"""

import sys

sys.path.insert(0, "/opt/trn_rl_repo")

import numpy as np
import ml_dtypes

from concourse import bass, bacc, tile, mybir
from concourse import ap_utils
from concourse.bass_utils import run_bass_kernel_spmd

P = 128
D = 64
NCORES = 8
ALPHA = 0.1
K_STEPS = 10
WRAP = 16  # dma_gather index wrap

LAST_RESULT = None  # test harness reads exec_time_ns from here


class Cfg:
    def __init__(self, n_nodes, nb, sec, ns_chunk):
        self.N = n_nodes
        self.NB = nb  # blocks (slots) per core
        self.SHARD = nb * P
        self.NPAD = NCORES * nb * P
        self.SEC = sec  # table section size (int16 index range)
        self.NSEC = (self.NPAD + sec - 1) // sec
        self.NS = ns_chunk  # slots per chunk
        assert self.NPAD % sec == 0


FULL = Cfg(100000, 98, 25088, 120)


def dma_gather_128(gp, out_ap, in_ap, idxs_ap, num_idxs, elem_size, elem_step):
    """nc.gpsimd.dma_gather minus the (transpose-only) elem%256B assert.

    Non-transpose, DRAM source. The Q7 ucode only requires the row *stride*
    (elem_step bytes) to be a multiple of 256.
    """
    assert idxs_ap.dtype == mybir.dt.int16
    assert in_ap.dtype == out_ap.dtype
    stride_bytes = elem_step * mybir.dt.size(in_ap.dtype)
    stride_bytes_256 = stride_bytes // 256
    assert stride_bytes_256 * 256 == stride_bytes and stride_bytes_256 < 256
    assert ap_utils.ap_is_contiguous(out_ap.ap[1:])
    assert ap_utils.ap_is_contiguous(idxs_ap.ap[1:])
    assert in_ap.ap[-1][1] == out_ap.ap[-1][1] == elem_size
    assert out_ap.ap[0][1] * out_ap.ap[1][1] == ((num_idxs + 127) // 128) * 128
    assert in_ap.ap[0][0] == elem_step
    _in_ap = gp.lower_ap_dma(in_ap, for_custom_bir_dma=True)
    _idxs_ap = gp.lower_ap(idxs_ap)
    _out_ap = gp.lower_ap(out_ap)
    return gp.add_instruction(
        mybir.InstDMAGatherAnt(
            name=gp.bass.get_next_instruction_name(),
            ins=[*_in_ap, _idxs_ap, gp.lower_val_access(gp.to_reg(num_idxs))],
            outs=[_out_ap],
            transpose=False,
            num_idxs=num_idxs,
            elem_size=elem_size,
            stride_bytes_256=stride_bytes_256,
            gen_mode=0,
            single_packet=False,
            queue_num=0,
            sbuf_tokens_per_rank=0,
            sbuf_free_dim_per_rank=0,
            sbuf_free_dim_pad_per_rank=0,
            sbuf_byte_offset=0,
        )
    )


def _preprocess(cfg, x, edge_row, edge_col, edge_weight):
    N, NB, SHARD, NPAD = cfg.N, cfg.NB, cfg.SHARD, cfg.NPAD
    SEC, NSEC, NS = cfg.SEC, cfg.NSEC, cfg.NS
    deg = np.bincount(edge_row, minlength=N)
    order = np.argsort(-deg, kind="stable").astype(np.int64)
    rows_sorted = np.concatenate([order, np.full(NPAD - N, -1, np.int64)])

    # snake assignment of degree-sorted blocks to cores
    block_of = np.empty((NCORES, NB), np.int64)
    for s in range(NB):
        base = s * NCORES
        for k in range(NCORES):
            block_of[k, s] = base + (k if s % 2 == 0 else NCORES - 1 - k)

    new_rows_old = np.empty(NPAD, np.int64)  # new position -> old id (-1 virtual)
    for k in range(NCORES):
        for s in range(NB):
            b = block_of[k, s]
            new_rows_old[k * SHARD + s * P : k * SHARD + s * P + P] = rows_sorted[b * P : (b + 1) * P]
    old_to_new = np.full(N, -1, np.int64)
    mask = new_rows_old >= 0
    old_to_new[new_rows_old[mask]] = np.nonzero(mask)[0]

    r_new = old_to_new[edge_row]
    c_new = old_to_new[edge_col]
    sec_e = (c_new // SEC).astype(np.int64)
    loc_e = (c_new % SEC).astype(np.int32)
    w = edge_weight.astype(np.float32)

    core_e = r_new // SHARD
    s_e = (r_new % SHARD) // P
    p_e = r_new % P

    # rank of edge within its (row, section) group
    gkey = r_new * NSEC + sec_e
    eorder = np.argsort(gkey, kind="stable")
    gk_s = gkey[eorder]
    starts = np.searchsorted(gk_s, gk_s)  # first index of each group value
    j_sorted = np.arange(len(gk_s)) - starts
    j_e = np.empty(len(gk_s), np.int64)
    j_e[eorder] = j_sorted

    # per-slot per-section max degree (over all cores & partitions)
    cnt = np.bincount(gkey, minlength=NPAD * NSEC).reshape(NCORES, NB, P, NSEC)
    g_slot = np.maximum(cnt.max(axis=(0, 2)), 1)  # [NB, NSEC]

    # greedy chunks of consecutive slots with ns * max_g(section) <= NS cap
    chunk_ns = []
    chunk_lo = []
    s0 = 0
    while s0 < NB:
        m = g_slot[s0].copy()
        s1 = s0 + 1
        while s1 < NB:
            m2 = np.maximum(m, g_slot[s1])
            if (s1 + 1 - s0) * m2.max() > NS:
                break
            m = m2
            s1 += 1
        chunk_lo.append(s0)
        chunk_ns.append(s1 - s0)
        s0 = s1
    nchunk = len(chunk_ns)
    k_of_slot = np.zeros(NB, np.int64)
    sloc_of_slot = np.zeros(NB, np.int64)
    for k in range(nchunk):
        for i in range(chunk_ns[k]):
            k_of_slot[chunk_lo[k] + i] = k
            sloc_of_slot[chunk_lo[k] + i] = i
    k_e = k_of_slot[s_e]
    sloc_e = sloc_of_slot[s_e]

    g_kc = np.ones((nchunk, NSEC), np.int64)
    for k in range(nchunk):
        g_kc[k] = g_slot[chunk_lo[k] : chunk_lo[k] + chunk_ns[k]].max(axis=0)

    # position space: ordered by (chunk, section, slot_in_chunk, j)
    base_kc = np.zeros((nchunk, NSEC), np.int64)
    tot = 0
    for k in range(nchunk):
        for c in range(NSEC):
            base_kc[k, c] = tot
            tot += chunk_ns[k] * g_kc[k, c]

    pos_e = base_kc[k_e, sec_e] + sloc_e * g_kc[k_e, sec_e] + j_e

    idx16 = np.zeros((NCORES, WRAP, tot * P // WRAP), np.int16)
    w_ell = np.zeros((NCORES, P, tot), ml_dtypes.bfloat16)
    i_e = pos_e * P + p_e  # linear index position (per instruction ordering holds globally)
    for k in range(NCORES):
        m = core_e == k
        w_ell[k][p_e[m], pos_e[m]] = w[m].astype(ml_dtypes.bfloat16)
        ii = i_e[m]
        idx16[k][ii % WRAP, ii // WRAP] = loc_e[m]
    idxw = np.ascontiguousarray(np.tile(idx16, (NCORES, 1)))  # [NCORES, 128, tot*8]

    # x arranged per core: [P, NB, D], pre-scaled by alpha/(1-alpha)
    x_new = np.zeros((NPAD, D), np.float32)
    x_new[mask] = x[new_rows_old[mask]]
    scale = ALPHA / (1.0 - ALPHA)
    x_ell = np.empty((NCORES, P, NB, D), np.float32)
    for k in range(NCORES):
        x_ell[k] = (x_new[k * SHARD : (k + 1) * SHARD] * scale).reshape(NB, P, D).transpose(1, 0, 2)

    h0 = np.zeros((NPAD, 2 * D), ml_dtypes.bfloat16)
    h0[:, :D] = x_new.astype(ml_dtypes.bfloat16)

    struct = (
        tuple(chunk_ns),
        tuple(tuple(int(g) for g in row) for row in g_kc),
        tuple(tuple(int(b) for b in row) for row in base_kc),
        int(tot),
        tuple(int(v) for v in chunk_lo),
    )
    return struct, idxw, w_ell, x_ell, h0, new_rows_old


def _build(cfg, struct):
    chunk_ns, g_kc, base_kc, tot, chunk_lo = struct
    NB, SHARD, NPAD, SEC, NSEC = cfg.NB, cfg.SHARD, cfg.NPAD, cfg.SEC, cfg.NSEC
    nchunk = len(chunk_ns)
    max_npos = max(
        chunk_ns[k] * g_kc[k][c] for k in range(nchunk) for c in range(NSEC)
    )
    max_ns = max(chunk_ns)

    nc = bacc.Bacc("TRN2", target_bir_lowering=False, debug=False, num_devices=NCORES)
    bf16, f32, i16 = mybir.dt.bfloat16, mybir.dt.float32, mybir.dt.int16

    idxw_in = nc.dram_tensor("idxw", [P, tot * 8], i16, kind="ExternalInput")
    w_in = nc.dram_tensor("w", [P, tot], bf16, kind="ExternalInput")
    x_in = nc.dram_tensor("x", [P, NB, D], f32, kind="ExternalInput")
    h0_in = nc.dram_tensor("h0", [NPAD, 2 * D], bf16, kind="ExternalInput")
    out_ext = nc.dram_tensor("out", [P, NB, D], f32, kind="ExternalOutput")

    tabA = nc.dram_tensor("tabA", [NPAD, 2 * D], bf16)
    tabB = nc.dram_tensor("tabB", [NPAD, 2 * D], bf16)
    sbA = nc.dram_tensor("sbA", [SHARD, 2 * D], bf16)
    sbB = nc.dram_tensor("sbB", [SHARD, 2 * D], bf16)

    with tile.TileContext(nc) as tc:
        with (
            tc.tile_pool(name="const", bufs=1) as cpool,
            tc.tile_pool(name="ix", bufs=2) as ipool,
            tc.tile_pool(name="gath", bufs=2) as gpool,
            tc.tile_pool(name="pp", bufs=2) as ppool,
            tc.tile_pool(name="red", bufs=1) as rpool,
        ):
            w_t = cpool.tile([P, tot], bf16, tag="w")
            x_t = cpool.tile([P, NB * D], f32, tag="x")
            hb2 = cpool.tile([P, NB * 2 * D], bf16, tag="hb2")
            nc.sync.dma_start(out=w_t[:], in_=w_in[:])
            nc.sync.dma_start(out=x_t[:], in_=x_in[:].rearrange("p b d -> p (b d)"))
            hb2v = hb2[:].rearrange("p (s e) -> p s e", e=2 * D)
            nc.gpsimd.memset(hb2v[:, :, D:], 0)

            tables = [h0_in]
            for t in range(K_STEPS - 1):
                tables.append(tabA if t % 2 == 0 else tabB)

            for t in range(K_STEPS):
                src = tables[t]
                red = rpool.tile([P, NB * D], f32, tag="red")
                for k in range(nchunk):
                    ns = chunk_ns[k]
                    pp = ppool.tile([P, max_ns * NSEC * D], f32, tag="pp")
                    for c in range(NSEC):
                        g = g_kc[k][c]
                        npos = ns * g
                        lo = base_kc[k][c]
                        it = ipool.tile([P, max_npos * 8], i16, tag="ix")
                        nc.sync.dma_start(
                            out=it[:, : npos * 8],
                            in_=idxw_in[:, lo * 8 : (lo + npos) * 8],
                        )
                        gt = gpool.tile([P, max_npos * D], bf16, tag="g")
                        dma_gather_128(
                            nc.gpsimd,
                            out_ap=gt[:, : npos * D].rearrange("p (n d) -> p n d", d=D),
                            in_ap=src[c * SEC : (c + 1) * SEC, :D],
                            idxs_ap=it[:, : npos * 8],
                            num_idxs=npos * P,
                            elem_size=D,
                            elem_step=2 * D,
                        )
                        wb = w_t[:, lo : lo + npos].unsqueeze(-1).to_broadcast([P, npos, D])
                        nc.vector.tensor_tensor(
                            out=gt[:, : npos * D].rearrange("p (n d) -> p n d", d=D),
                            in0=gt[:, : npos * D].rearrange("p (n d) -> p n d", d=D),
                            in1=wb,
                            op=mybir.AluOpType.mult,
                        )
                        # reduce over j: [P, ns, D, g] -> pp[:, :, c, :]
                        seg = gt[:, : npos * D].rearrange("p (s g d) -> p s d g", g=g, d=D)
                        ppv = pp[:, : ns * NSEC * D].rearrange(
                            "p (s c d) -> p s c d", c=NSEC, d=D
                        )[:, :, c, :]
                        nc.vector.tensor_reduce(
                            out=ppv, in_=seg, axis=mybir.AxisListType.X, op=mybir.AluOpType.add
                        )
                    # combine sections: [P, ns, D, NSEC] -> red slot range
                    s0 = chunk_lo[k]
                    nc.vector.tensor_reduce(
                        out=red[:, s0 * D : (s0 + ns) * D],
                        in_=pp[:, : ns * NSEC * D].rearrange(
                            "p (s c d) -> p s d c", c=NSEC, d=D
                        ),
                        axis=mybir.AxisListType.X,
                        op=mybir.AluOpType.add,
                    )
                # h_{t+1} = 0.9 * (red + x/9)
                nc.vector.tensor_tensor(
                    out=red[:], in0=red[:], in1=x_t[:], op=mybir.AluOpType.add
                )
                if t < K_STEPS - 1:
                    nc.vector.tensor_scalar_mul(
                        out=hb2v[:, :, :D],
                        in0=red[:].rearrange("p (s d) -> p s d", d=D),
                        scalar1=1.0 - ALPHA,
                    )
                    sb = sbA if t % 2 == 0 else sbB
                    nc.sync.dma_start(
                        out=sb[:].rearrange("(s p) e -> p s e", p=P),
                        in_=hb2v,
                    )
                    nc.gpsimd.collective_compute(
                        "AllGather",
                        mybir.AluOpType.bypass,
                        replica_groups=[list(range(NCORES))],
                        ins=[sb.ap().opt()],
                        outs=[tables[t + 1].ap().opt()],
                    )
                else:
                    nc.vector.tensor_scalar_mul(out=red[:], in0=red[:], scalar1=1.0 - ALPHA)
                    nc.sync.dma_start(
                        out=out_ext[:].rearrange("p b d -> p (b d)"), in_=red[:]
                    )
    nc.compile()
    return nc


_BUILD_CACHE = {}


def _kernel_impl(cfg, x, edge_row, edge_col, edge_weight, trace=False):
    global LAST_RESULT
    struct, idxw, w_ell, x_ell, h0, new_rows_old = _preprocess(
        cfg, x, edge_row, edge_col, edge_weight
    )
    key = (cfg.N, struct[0], struct[1])
    if key not in _BUILD_CACHE:
        _BUILD_CACHE[key] = _build(cfg, struct)
    nc = _BUILD_CACHE[key]

    in_maps = [
        {"idxw": idxw[k], "w": w_ell[k], "x": x_ell[k], "h0": h0}
        for k in range(NCORES)
    ]
    res = run_bass_kernel_spmd(nc, in_maps, core_ids=list(range(NCORES)), trace=trace)
    LAST_RESULT = res

    SHARD = cfg.SHARD
    full_new = np.empty((cfg.NPAD, D), np.float32)
    for k in range(NCORES):
        o = np.asarray(res.results[k]["out"]).reshape(P, cfg.NB, D)
        full_new[k * SHARD : (k + 1) * SHARD] = o.transpose(1, 0, 2).reshape(SHARD, D)
    out = np.empty((cfg.N, D), np.float32)
    mask = new_rows_old >= 0
    out[new_rows_old[mask]] = full_new[mask]
    return out


def kernel(x, edge_row, edge_col, edge_weight, _trace=False):
    x = np.asarray(x, dtype=np.float32)
    edge_row = np.asarray(edge_row, dtype=np.int32)
    edge_col = np.asarray(edge_col, dtype=np.int32)
    edge_weight = np.asarray(edge_weight, dtype=np.float32)
    return _kernel_impl(FULL, x, edge_row, edge_col, edge_weight, trace=_trace)
